# revision 1
# baseline (speedup 1.0000x reference)
"""Trainium2 Bass kernel: ragged GQA flash-decode attention (B=16, Hq=32, Hkv=8, D=128).

Strategy (SPMD over 8 NeuronCores, data-parallel over 128-slot KV tiles):
  host: scatter the step's new K/V token into its slot while packing each
        sequence's valid KV prefix into 128-slot tiles; distribute the global
        tile list evenly over the 8 cores (a sequence may span cores --
        flash-decoding style chunking).
  device, per tile: one contiguous 1 MiB KV DMA; per kv-head PE-transpose of
        K (fp32, via identity); scores^T [s, G] = (K^T).T @ Q^T with the slot
        axis on partitions; no-max softmax (scores ~ N(0,1)) as a single
        ACT exp(scale*x + bias) where bias in {0, -1e30} masks padding slots;
        l = ones.T @ P on the PE; O^T = V.T @ P (V used in natural layout).
        Per-tile partials (O^T [128,32], l [32]) go back to DRAM.
  host: sum partials per sequence in fp64, divide by l, transpose.
"""

import math
from contextlib import ExitStack

import numpy as np

N_CORES = 8
B, HQ, HKV, D = 16, 32, 8, 128
G = HQ // HKV
ROW = 2 * HKV * D  # 2048 floats per kv_buffer row
SCALE = 1.0 / math.sqrt(D)
NEG = -1.0e30

_COMPILED: dict = {}


def _build_program(T: int, niter: int = 1):
    """Build + compile the SPMD program for T tiles per core.

    Returns (nc, meta) where meta holds the IO names/avals needed to run it.
    niter > 1 wraps the whole per-tile pipeline in a hardware For_i loop so
    test harnesses can measure steady-state HW time by delta-timing.
    """
    import concourse.mybir as mybir
    import concourse.tile as tile
    from concourse import bacc
    from concourse.masks import make_identity

    f32 = mybir.dt.float32
    nc = bacc.Bacc("TRN2", target_bir_lowering=False, debug=False, num_devices=N_CORES)

    kv = nc.dram_tensor("kv", [T * 128, ROW], f32, kind="ExternalInput").ap()
    qt = nc.dram_tensor("qt", [128, 32 * T], f32, kind="ExternalInput").ap()
    bias = nc.dram_tensor("bias", [128, T], f32, kind="ExternalInput").ap()
    o = nc.dram_tensor("o", [128, 32 * T], f32, kind="ExternalOutput").ap()
    lo = nc.dram_tensor("l", [1, 32 * T], f32, kind="ExternalOutput").ap()

    with tile.TileContext(nc) as tc, ExitStack() as ctx:
        kv_pool = ctx.enter_context(tc.tile_pool(name="kv", bufs=4))
        ktp_pool = ctx.enter_context(tc.tile_pool(name="ktp", bufs=3, space="PSUM"))
        kt_pool = ctx.enter_context(tc.tile_pool(name="kt", bufs=12))
        sc_pool = ctx.enter_context(tc.tile_pool(name="sc", bufs=2, space="PSUM"))
        p_pool = ctx.enter_context(tc.tile_pool(name="p", bufs=3))
        ol_pool = ctx.enter_context(tc.tile_pool(name="ol", bufs=2, space="PSUM"))
        const_pool = ctx.enter_context(tc.tile_pool(name="const", bufs=1))
        io_pool = ctx.enter_context(tc.tile_pool(name="io", bufs=1))

        ident = const_pool.tile([128, 128], f32)
        make_identity(nc, ident[:])
        ones = const_pool.tile([128, 1], f32)
        nc.gpsimd.memset(ones[:], 1.0)

        qt_s = io_pool.tile([128, 32 * T], f32)
        nc.sync.dma_start(qt_s[:], qt)
        bias_s = io_pool.tile([128, T], f32)
        nc.sync.dma_start(bias_s[:], bias)
        o_all = io_pool.tile([128, 32 * T], f32)
        l_all = io_pool.tile([1, 32 * T], f32)

        def body():
            for t in range(T):
                kvt = kv_pool.tile([128, ROW], f32)
                nc.sync.dma_start(kvt[:], kv[t * 128:(t + 1) * 128, :])
                kts = []
                for h in range(HKV):
                    ktp = ktp_pool.tile([128, 128], f32)
                    nc.tensor.transpose(
                        ktp[:], kvt[:, h * 128:(h + 1) * 128], ident[:]
                    )
                    kt = kt_pool.tile([128, 128], f32)
                    nc.vector.tensor_copy(kt[:], ktp[:])
                    kts.append(kt)
                sc = sc_pool.tile([128, 32], f32)
                for h in range(HKV):
                    nc.tensor.matmul(
                        sc[:, h * G:(h + 1) * G],
                        kts[h][:],
                        qt_s[:, 32 * t + h * G:32 * t + (h + 1) * G],
                        start=True,
                        stop=True,
                    )
                p = p_pool.tile([128, 32], f32)
                nc.scalar.activation(
                    p[:],
                    sc[:],
                    mybir.ActivationFunctionType.Exp,
                    bias=bias_s[:, t:t + 1],
                    scale=SCALE,
                )
                ol = ol_pool.tile([128, 64], f32)
                nc.tensor.matmul(ol[0:1, 32:64], ones[:], p[:], start=True, stop=True)
                for h in range(HKV):
                    nc.tensor.matmul(
                        ol[:, h * G:(h + 1) * G],
                        kvt[:, 1024 + h * 128:1024 + (h + 1) * 128],
                        p[:, h * G:(h + 1) * G],
                        start=True,
                        stop=True,
                    )
                nc.vector.tensor_copy(o_all[:, 32 * t:32 * t + 32], ol[:, 0:32])
                nc.vector.tensor_copy(l_all[0:1, 32 * t:32 * t + 32], ol[0:1, 32:64])

        if niter > 1:
            with tc.For_i(0, niter, 1):
                body()
        else:
            body()

        nc.sync.dma_start(o, o_all[:])
        nc.sync.dma_start(lo, l_all[:])

    nc.compile()
    return nc


def _make_runner(nc):
    """Build a persistent jitted SPMD runner for a compiled Bacc program.

    Mirrors concourse.bass2jax.run_bass_via_pjrt (the axon path of
    run_bass_kernel_spmd) but keeps the jitted callable so repeat calls
    don't re-trace. Returns run(concat_inputs: dict[str, np.ndarray]) ->
    dict[str, np.ndarray] of concatenated (n_cores*dim0, ...) outputs.
    """
    import jax
    import concourse.mybir as mybir
    from jax.experimental.shard_map import shard_map
    from jax.sharding import Mesh, PartitionSpec

    from concourse.bass2jax import (
        _bass_exec_p,
        install_neuronx_cc_hook,
        partition_id_tensor,
    )

    install_neuronx_cc_hook()

    partition_name = nc.partition_id_tensor.name if nc.partition_id_tensor else None
    in_names, out_names, out_avals, zero_shapes = [], [], [], []
    for alloc in nc.m.functions[0].allocations:
        if not isinstance(alloc, mybir.MemoryLocationSet):
            continue
        name = alloc.memorylocations[0].name
        if alloc.kind == "ExternalInput":
            if name != partition_name:
                in_names.append(name)
        elif alloc.kind == "ExternalOutput":
            out_names.append(name)
            shape = tuple(alloc.tensor_shape)
            dtype = mybir.dt.np(alloc.dtype)
            out_avals.append(jax.core.ShapedArray(shape, dtype))
            zero_shapes.append((shape, dtype))
    n_params = len(in_names)
    n_outs = len(out_avals)
    all_in_names = list(in_names) + list(out_names)
    if partition_name is not None:
        all_in_names.append(partition_name)

    def _body(*args):
        operands = list(args)
        if partition_name is not None:
            operands.append(partition_id_tensor())
        outs = _bass_exec_p.bind(
            *operands,
            out_avals=tuple(out_avals),
            in_names=tuple(all_in_names),
            out_names=tuple(out_names),
            lowering_input_output_aliases=(),
            sim_require_finite=True,
            sim_require_nnan=True,
            nc=nc,
        )
        return tuple(outs)

    devices = jax.devices()[:N_CORES]
    assert len(devices) >= N_CORES, f"need {N_CORES} devices, have {len(devices)}"
    mesh = Mesh(np.asarray(devices[:N_CORES]), ("core",))
    in_specs = (PartitionSpec("core"),) * (n_params + n_outs)
    out_specs = (PartitionSpec("core"),) * n_outs
    donate = tuple(range(n_params, n_params + n_outs))
    sharded = jax.jit(
        shard_map(
            _body, mesh=mesh, in_specs=in_specs, out_specs=out_specs, check_rep=False
        ),
        donate_argnums=donate,
        keep_unused=True,
    )

    def run(concat_inputs):
        args = [concat_inputs[name] for name in in_names]
        zeros = [
            np.zeros((N_CORES * s[0], *s[1:]), d) for (s, d) in zero_shapes
        ]
        out_arrs = sharded(*args, *zeros)
        out_arrs = [np.asarray(a) for a in out_arrs]
        return {name: out_arrs[i] for i, name in enumerate(out_names)}

    run.in_names = in_names
    run.out_names = out_names
    run.out_avals = out_avals
    run.zero_shapes = zero_shapes
    run.sharded = sharded
    run.mesh = mesh
    return run


def _plan(b_seq_len):
    """Global tile list [(b, j)] and per-core layout. Returns (entries, T)
    where entries has length 8*T, padded with (-1, -1)."""
    lens = [int(x) for x in b_seq_len]
    entries = []
    for b, ln in enumerate(lens):
        for j in range((ln + 127) // 128):
            entries.append((b, j))
    T = (len(entries) + N_CORES - 1) // N_CORES
    entries += [(-1, -1)] * (N_CORES * T - len(entries))
    return entries, T


def _pack(xq, xk, xv, kv_buffer, cur_select_index, start_index, b_seq_len, entries, T):
    lens = np.asarray(b_seq_len, dtype=np.int64)
    starts = np.asarray(start_index, dtype=np.int64)
    csi = np.asarray(cur_select_index, dtype=np.int64)

    kv_all = np.zeros((N_CORES * T * 128, ROW), dtype=np.float32)
    qt_all = np.zeros((N_CORES * 128, 32 * T), dtype=np.float32)
    bias_all = np.full((N_CORES * 128, T), NEG, dtype=np.float32)

    kvb = np.asarray(kv_buffer).reshape(-1, ROW)
    new_kv = np.concatenate(
        [np.asarray(xk)[:, 0], np.asarray(xv)[:, 0]], axis=1
    ).reshape(B, ROW)  # [B, 2*HKV*D]
    qts = np.asarray(xq)[:, 0].transpose(0, 2, 1).astype(np.float32)  # [B, D, HQ]

    for i, (b, j) in enumerate(entries):
        if b < 0:
            continue
        c, t = divmod(i, T)
        r0 = (c * T + t) * 128
        src0 = int(starts[b]) + j * 128
        kv_all[r0:r0 + 128] = kvb[src0:src0 + 128]
        sel = int(csi[b])
        if src0 <= sel < src0 + 128:
            kv_all[r0 + (sel - src0)] = new_kv[b]
        qt_all[c * 128:(c + 1) * 128, 32 * t:32 * t + 32] = qts[b]
        nvalid = min(128, int(lens[b]) - j * 128)
        bias_all[c * 128:c * 128 + nvalid, t] = 0.0
    return {"kv": kv_all, "qt": qt_all, "bias": bias_all}


def _combine(o_cat, l_cat, entries, T):
    acc = np.zeros((B, D, HQ), dtype=np.float64)
    lacc = np.zeros((B, HQ), dtype=np.float64)
    o_cat = o_cat.reshape(N_CORES, 128, 32 * T)
    l_cat = l_cat.reshape(N_CORES, 1, 32 * T)
    for i, (b, j) in enumerate(entries):
        if b < 0:
            continue
        c, t = divmod(i, T)
        acc[b] += o_cat[c, :, 32 * t:32 * t + 32]
        lacc[b] += l_cat[c, 0, 32 * t:32 * t + 32]
    out = (acc / lacc[:, None, :]).transpose(0, 2, 1)  # [B, HQ, D]
    return out.reshape(B, 1, HQ * D).astype(np.float32)


def get_compiled(T, niter=1):
    key = (T, niter)
    if key not in _COMPILED:
        nc = _build_program(T, niter)
        _COMPILED[key] = _make_runner(nc)
    return _COMPILED[key]


def kernel(xq, xk, xv, kv_buffer, cur_select_index, start_index, b_seq_len,
           max_actual_seq_len=None):
    entries, T = _plan(b_seq_len)
    inputs = _pack(xq, xk, xv, kv_buffer, cur_select_index, start_index,
                   b_seq_len, entries, T)
    run = get_compiled(T)
    outs = run(inputs)
    return _combine(outs["o"], outs["l"], entries, T)
